# revision 14
# baseline (speedup 1.0000x reference)
"""Trainium2 Bass kernel for YOLO-style detection decode (nms_detection).

Computes, for input `output` (B=8, H=80, W=80, A*85=255):
  per (b, cell, anchor):  xy = (sigmoid(txy) + grid_off) * stride
                          wh = exp(twh) * anchor
                          bbox = [xy - wh/2, xy + wh/2]
                          p_c = sigmoid(cls_c) * sigmoid(obj)
  out (B, C*hw*A, 6) rows = [cid, score, x1, y1, x2, y2] where
  cid = c if p_c > 0.01 else -1, score = p_c if p_c > 0.01 else 0.

Sharding: pure data parallel over batch, one batch element per NeuronCore.

Per-core design (output is 37 MB/core -> store-bandwidth bound):
  - fully CELL-MAJOR pipeline: partition p owns q consecutive cells of each
    128*q-cell supertile. No transposes, no PSUM, no TensorE at all; every op
    runs on all 128 partitions.
  - output staging tiles are [128, 40, q, A, 6] (class in the FREE dim, two
    40-class halves); the store DMA's DRAM-side AP (p, c, k) =
    c*115200 + c0*18 + p*q*18 + k writes q*72-byte contiguous runs per
    (partition, class) - all 16 SDMA engines carry equal load.
  - supertile schedule [4, 14, 16, 16]*128 cells: the small first tile gets
    the first store in flight early; the big tiles give 1008/1152B DMA
    descriptors (>=512B line-rate).
  - the two class-halves use bufs=1 tiles: store(half, st) overlaps
    assembly of the other half / next supertile.
  - score & cid each use one fused scalar_tensor_tensor:
      score = (S > t) * S;  cid+1 = (S > t) * (c+1), then ScalarE adds -1.
  - bbox columns are broadcast across classes with free-dim stride-0 APs,
    split between DVE (2 elem/cyc copies) and ScalarE.
  - exp(x) = sigmoid(x)/sigmoid(-x) so ScalarE never switches tables.
"""

import sys
import os
from contextlib import ExitStack

if "/opt/trn_rl_repo" not in sys.path:
    sys.path.insert(0, "/opt/trn_rl_repo")

import numpy as np

NUM_CLASSES = 80
NUM_ANCHOR = 3
NUM_PRED = 85
HW_CELLS = 6400
THRESH = 0.01
N_CORES = 8
ROW = 6 * NUM_ANCHOR  # f32 per cell per class in the output (18)

# cells-per-partition for each supertile; sum must be HW_CELLS/128 = 50
QS = tuple(int(x) for x in os.environ.get("KERNEL_QS", "4,14,16,16").split(","))
assert sum(QS) == HW_CELLS // 128

CHALF = NUM_CLASSES // 2  # classes per store half (40)
# within each half, classes [0, BSP) go to DVE, [BSP, CHALF) to ScalarE
BSP = int(os.environ.get("KERNEL_BSP", "22"))

_CACHE = {}
LAST_RESULT = None  # BassKernelResults of the most recent kernel() call


def _build(stride_f: float):
    import concourse.bass as bass  # noqa: F401
    import concourse.bacc as bacc
    import concourse.tile as tile
    from concourse import mybir

    f32 = mybir.dt.float32
    AF = mybir.ActivationFunctionType
    OP = mybir.AluOpType

    C = NUM_CLASSES
    A = NUM_ANCHOR

    # consts blob: [offs (50*A*2) | hanch (A*2) | cpat (C)]
    OFF_HANCH = 50 * A * 2         # 300
    OFF_CPAT = OFF_HANCH + A * 2   # 306
    CONST_F = OFF_CPAT + C         # 386

    nc = bacc.Bacc("TRN2", target_bir_lowering=False, debug=False)
    x_d = nc.declare_dram_parameter("x", [HW_CELLS, A * NUM_PRED], f32, isOutput=False)
    const_d = nc.declare_dram_parameter("consts", [128, CONST_F], f32, isOutput=False)
    out_d = nc.declare_dram_parameter("out", [C, HW_CELLS * ROW], f32, isOutput=True)

    with ExitStack() as ctx:
        tc = ctx.enter_context(tile.TileContext(nc))
        cpool = ctx.enter_context(tc.tile_pool(name="const", bufs=1))
        in_pool = ctx.enter_context(tc.tile_pool(name="inp", bufs=2))
        sm_pool = ctx.enter_context(tc.tile_pool(name="small", bufs=2))
        s_pool = ctx.enter_context(tc.tile_pool(name="scls", bufs=2))
        oa_pool = ctx.enter_context(tc.tile_pool(name="outa", bufs=1))
        ob_pool = ctx.enter_context(tc.tile_pool(name="outb", bufs=1))

        # ---- constants (one DMA -> one sem lane) ----
        const_sb = cpool.tile([128, CONST_F], f32, tag="consts")
        nc.gpsimd.dma_start(out=const_sb[:, :], in_=const_d[:, :])
        hanch_v = const_sb[:, OFF_HANCH:OFF_CPAT].rearrange(
            "p (u a k) -> p u a k", a=A, k=2
        )
        cpat_v = const_sb[:, OFF_CPAT:CONST_F].rearrange("p (c u) -> p c u", u=1)

        # ---- warm-up: let each engine observe the const DMA once, so no
        # later instruction needs more than one sync-wait (ISA limit) ----
        warm = cpool.tile([128, 4], f32, tag="warm")
        nc.vector.tensor_copy(warm[0:1, 0:1], const_sb[0:1, 0:1])
        nc.scalar.copy(warm[0:1, 1:2], const_sb[0:1, 0:1])
        nc.gpsimd.tensor_copy(warm[0:1, 2:3], const_sb[0:1, 0:1])

        qoff = 0
        for st, q in enumerate(QS):
            cells = 128 * q
            c0 = 128 * qoff  # starting cell = partition0's first cell offset... (layout below)

            # ---- load: partition p holds cells [c0 + q*p, c0 + q*p + q) ----
            in_t = in_pool.tile([128, QS[-1], A * NUM_PRED], f32, tag="in")
            nc.gpsimd.dma_start(
                out=in_t[:, 0:q, :],
                in_=x_d[c0 : c0 + cells, :].rearrange("(p q) c -> p q c", p=128),
            )
            in_v = in_t[:, 0:q, :].rearrange("p q (a c) -> p q a c", a=A, c=NUM_PRED)

            # exp(wh) = sigmoid(wh)/sigmoid(-wh); sgnw reads raw wh so it runs
            # before the in-place sigmoid (same engine keeps them ordered)
            sgnw = sm_pool.tile([128, QS[-1], A, 2], f32, tag="sgnw")
            nc.scalar.activation(
                sgnw[:, 0:q, :, :], in_v[:, :, :, 2:4], AF.Sigmoid, scale=-1.0
            )
            # sigmoid of everything, in place
            nc.scalar.activation(in_t[:, 0:q, :], in_t[:, 0:q, :], AF.Sigmoid)
            sig_v = in_v

            rec = sm_pool.tile([128, QS[-1], A, 2], f32, tag="rec")
            nc.vector.reciprocal(rec[:, 0:q, :, :], sgnw[:, 0:q, :, :])
            t1 = sm_pool.tile([128, QS[-1], A, 2], f32, tag="t1")
            nc.gpsimd.tensor_tensor(
                t1[:, 0:q, :, :],
                sig_v[:, :, :, 2:4],
                hanch_v.to_broadcast([128, q, A, 2]),
                OP.mult,
            )
            halfwh = sm_pool.tile([128, QS[-1], A, 2], f32, tag="halfwh")
            nc.gpsimd.tensor_tensor(
                halfwh[:, 0:q, :, :], t1[:, 0:q, :, :], rec[:, 0:q, :, :], OP.mult
            )

            # xy = sigmoid(xy)*stride + off*stride
            xy = sm_pool.tile([128, QS[-1], A, 2], f32, tag="xy")
            nc.vector.scalar_tensor_tensor(
                xy[:, 0:q, :, :],
                in0=sig_v[:, :, :, 0:2],
                scalar=stride_f,
                in1=const_sb[:, qoff * A * 2 : (qoff + q) * A * 2].rearrange(
                    "p (q a k) -> p q a k", a=A, k=2
                ),
                op0=OP.mult,
                op1=OP.add,
            )

            # bbox cell-major: [p, 1, q, a, 0:2]=xy-halfwh, [2:4]=xy+halfwh
            # (GpSimd: keeps DVE free for the score/cid/bbox-scatter work)
            bb = sm_pool.tile([128, 1, QS[-1], A, 4], f32, tag="bb")
            nc.gpsimd.tensor_tensor(
                bb[:, 0, 0:q, :, 0:2], xy[:, 0:q, :, :], halfwh[:, 0:q, :, :], OP.subtract
            )
            nc.gpsimd.tensor_tensor(
                bb[:, 0, 0:q, :, 2:4], xy[:, 0:q, :, :], halfwh[:, 0:q, :, :], OP.add
            )

            # class scores S[p, q, a, c] = sigmoid(cls) * sigmoid(obj)
            S = s_pool.tile([128, QS[-1], A, C], f32, tag="S")
            nc.gpsimd.tensor_tensor(
                S[:, 0:q, :, :],
                sig_v[:, :, :, 5:85],
                sig_v[:, :, :, 4:5].to_broadcast([128, q, A, C]),
                OP.mult,
            )
            S_cqa = S[:, 0:q, :, :].rearrange("p q a c -> p c (q a)")

            # ---- per class-half: assemble + store ----
            for h, pool in ((0, oa_pool), (1, ob_pool)):
                cl = h * CHALF
                ov = pool.tile([128, CHALF, QS[-1], A, 6], f32, tag=f"ov{h}")
                ov_col = ov[:, :, 0:q, :, :].rearrange("p c q a e -> p c (q a) e")

                # score = (S > t) * S
                nc.vector.scalar_tensor_tensor(
                    ov_col[:, :, :, 1],
                    in0=S_cqa[:, cl : cl + CHALF, :],
                    scalar=THRESH,
                    in1=S_cqa[:, cl : cl + CHALF, :],
                    op0=OP.is_gt,
                    op1=OP.mult,
                )
                # cid+1 = (S > t) * (c+1); ScalarE applies the -1
                nc.vector.scalar_tensor_tensor(
                    ov_col[:, :, :, 0],
                    in0=S_cqa[:, cl : cl + CHALF, :],
                    scalar=THRESH,
                    in1=cpat_v[:, cl : cl + CHALF, :].to_broadcast([128, CHALF, q * A]),
                    op0=OP.is_gt,
                    op1=OP.mult,
                )
                nc.scalar.activation(
                    ov_col[:, :, :, 0], ov_col[:, :, :, 0], AF.Copy, bias=-1.0
                )

                # bbox broadcast across classes, per anchor, DVE/ScalarE split
                for a in range(A):
                    src = bb[:, :, 0:q, a, :]
                    nc.vector.tensor_copy(
                        ov[:, 0:BSP, 0:q, a, 2:6],
                        src.to_broadcast([128, BSP, q, 4]),
                    )
                    nc.scalar.copy(
                        ov[:, BSP:CHALF, 0:q, a, 2:6],
                        src.to_broadcast([128, CHALF - BSP, q, 4]),
                    )

                # store: DRAM (p, c, k) = (cl+c)*115200 + c0*18 + p*q*18 + k
                nc.sync.dma_start(
                    out=out_d[
                        cl : cl + CHALF, c0 * ROW : (c0 + cells) * ROW
                    ].rearrange("c (p k) -> p c k", p=128),
                    in_=ov[:, :, 0:q, :, :].rearrange("p c q a e -> p c (q a e)"),
                )
            qoff += q

    nc.finalize()
    return nc


def make_consts(anchor, offset, stride_f):
    """Pack [offs | hanch | cpat] into one (128, F) f32 blob."""
    off = np.asarray(offset, dtype=np.float32).reshape(-1, 2)[:HW_CELLS] * stride_f
    # offs[p, qoff+i, a, k] = off[c0 + p*q + i, k] for supertile at (c0, q)
    cols = []
    c0 = 0
    for q in QS:
        o = off[c0 : c0 + 128 * q].reshape(128, q, 1, 2)
        cols.append(np.broadcast_to(o, (128, q, NUM_ANCHOR, 2)).reshape(128, -1))
        c0 += 128 * q
    offs_cols = np.ascontiguousarray(np.concatenate(cols, axis=1))
    a2 = np.asarray(anchor, dtype=np.float32).reshape(NUM_ANCHOR * 2)
    hanch = np.tile(a2 / 2.0, (128, 1)).astype(np.float32)   # (128, 6)
    cpat = np.tile(np.arange(1, NUM_CLASSES + 1, dtype=np.float32), (128, 1))
    blob = np.concatenate([offs_cols, hanch, cpat], axis=1)
    return np.ascontiguousarray(blob.astype(np.float32))


def _host_prep(output, anchor, offset, stride):
    stride_f = float(stride)
    B = output.shape[0]
    x_all = np.ascontiguousarray(
        np.asarray(output, dtype=np.float32).reshape(B, HW_CELLS, NUM_ANCHOR * NUM_PRED)
    )
    consts = make_consts(anchor, offset, stride_f)
    return stride_f, x_all, consts


def kernel(output, anchor, offset, stride):
    from concourse.bass_utils import run_bass_kernel_spmd

    stride_f, x_all, consts = _host_prep(output, anchor, offset, stride)
    key = ("nc", stride_f)
    if key not in _CACHE:
        _CACHE[key] = _build(stride_f)
    nc = _CACHE[key]

    in_maps = [{"x": x_all[b], "consts": consts} for b in range(N_CORES)]
    res = run_bass_kernel_spmd(
        nc,
        in_maps,
        list(range(N_CORES)),
        tmpdir=os.environ.get("KERNEL_TRACE_DIR") or None,
    )
    global LAST_RESULT
    LAST_RESULT = res
    outs = [
        r["out"].reshape(NUM_CLASSES * HW_CELLS * NUM_ANCHOR, 6) for r in res.results
    ]
    return np.stack(outs, axis=0)


if __name__ == "__main__":
    rng = np.random.default_rng(0)
    out = rng.standard_normal((8, 80, 80, 255), dtype=np.float32)
    anchor = rng.uniform(10.0, 120.0, (1, 1, 3, 2)).astype(np.float32)
    gy, gx = np.meshgrid(np.arange(80, dtype=np.float32), np.arange(80, dtype=np.float32), indexing="ij")
    offset = np.stack([gx, gy], axis=-1).reshape(1, 80, 80, 1, 2)
    r = kernel(out, anchor, offset, 8)
    print(r.shape, r.dtype)


# revision 16
# speedup vs baseline: 1.2228x; 1.2228x over previous
"""Trainium2 Bass kernel for YOLO-style detection decode (nms_detection).

Computes, for input `output` (B=8, H=80, W=80, A*85=255):
  per (b, cell, anchor):  xy = (sigmoid(txy) + grid_off) * stride
                          wh = exp(twh) * anchor
                          bbox = [xy - wh/2, xy + wh/2]
                          p_c = sigmoid(cls_c) * sigmoid(obj)
  out (B, C*hw*A, 6) rows = [cid, score, x1, y1, x2, y2] where
  cid = c if p_c > 0.01 else -1, score = p_c if p_c > 0.01 else 0.

Sharding: pure data parallel over batch, one batch element per NeuronCore.

Per-core design (output is 37 MB/core -> store-bandwidth bound):
  - fully CELL-MAJOR pipeline: partition p owns q consecutive cells of each
    128*q-cell supertile. No transposes, no PSUM, no TensorE at all; every op
    runs on all 128 partitions.
  - output staging tiles are [128, 40, q, A, 6] (class in the FREE dim, two
    40-class halves); the store DMA's DRAM-side AP (p, c, k) =
    c*115200 + c0*18 + p*q*18 + k writes q*72-byte contiguous runs per
    (partition, class) - all 16 SDMA engines carry equal load.
  - supertile schedule [4, 14, 16, 16]*128 cells: the small first tile gets
    the first store in flight early; the big tiles give 1008/1152B DMA
    descriptors (>=512B line-rate).
  - the two class-halves use bufs=1 tiles: store(half, st) overlaps
    assembly of the other half / next supertile.
  - score & cid each use one fused scalar_tensor_tensor:
      score = (S > t) * S;  cid+1 = (S > t) * (c+1), then ScalarE adds -1.
  - bbox columns are broadcast across classes with free-dim stride-0 APs,
    split between DVE (2 elem/cyc copies) and ScalarE.
  - exp(x) = sigmoid(x)/sigmoid(-x) so ScalarE never switches tables.
"""

import sys
import os
from contextlib import ExitStack

if "/opt/trn_rl_repo" not in sys.path:
    sys.path.insert(0, "/opt/trn_rl_repo")

import numpy as np

NUM_CLASSES = 80
NUM_ANCHOR = 3
NUM_PRED = 85
HW_CELLS = 6400
THRESH = 0.01
N_CORES = 8
ROW = 6 * NUM_ANCHOR  # f32 per cell per class in the output (18)

# cells-per-partition for each supertile; sum must be HW_CELLS/128 = 50
QS = tuple(int(x) for x in os.environ.get("KERNEL_QS", "4,14,16,16").split(","))
assert sum(QS) == HW_CELLS // 128

CHALF = NUM_CLASSES // 2  # classes per store half (40)
# within each half, classes [0, BSP) go to DVE, [BSP, CHALF) to ScalarE
BSP = int(os.environ.get("KERNEL_BSP", "22"))

_CACHE = {}
LAST_RESULT = None  # BassKernelResults of the most recent kernel() call


def _build(stride_f: float):
    import concourse.bass as bass  # noqa: F401
    import concourse.bacc as bacc
    import concourse.tile as tile
    from concourse import mybir

    f32 = mybir.dt.float32
    AF = mybir.ActivationFunctionType
    OP = mybir.AluOpType

    C = NUM_CLASSES
    A = NUM_ANCHOR

    # consts blob: [offs (50*A*2) | hanch (A*2) | cpat (C)]
    OFF_HANCH = 50 * A * 2         # 300
    OFF_CPAT = OFF_HANCH + A * 2   # 306
    CONST_F = OFF_CPAT + C         # 386

    nc = bacc.Bacc("TRN2", target_bir_lowering=False, debug=False)
    x_d = nc.declare_dram_parameter("x", [HW_CELLS, A * NUM_PRED], f32, isOutput=False)
    const_d = nc.declare_dram_parameter("consts", [128, CONST_F], f32, isOutput=False)
    out_d = nc.declare_dram_parameter("out", [C, HW_CELLS * ROW], f32, isOutput=True)

    with ExitStack() as ctx:
        tc = ctx.enter_context(tile.TileContext(nc))
        cpool = ctx.enter_context(tc.tile_pool(name="const", bufs=1))
        in_pool = ctx.enter_context(tc.tile_pool(name="inp", bufs=2))
        sm_pool = ctx.enter_context(tc.tile_pool(name="small", bufs=2))
        s_pool = ctx.enter_context(tc.tile_pool(name="scls", bufs=2))
        oa_pool = ctx.enter_context(tc.tile_pool(name="outa", bufs=1))
        ob_pool = ctx.enter_context(tc.tile_pool(name="outb", bufs=1))

        # ---- constants (one DMA -> one sem lane) ----
        const_sb = cpool.tile([128, CONST_F], f32, tag="consts")
        nc.gpsimd.dma_start(out=const_sb[:, :], in_=const_d[:, :])
        hanch_v = const_sb[:, OFF_HANCH:OFF_CPAT].rearrange(
            "p (u a k) -> p u a k", a=A, k=2
        )
        cpat_v = const_sb[:, OFF_CPAT:CONST_F].rearrange("p (c u) -> p c u", u=1)

        # ---- warm-up: let each engine observe the const DMA once, so no
        # later instruction needs more than one sync-wait (ISA limit) ----
        warm = cpool.tile([128, 4], f32, tag="warm")
        nc.vector.tensor_copy(warm[0:1, 0:1], const_sb[0:1, 0:1])
        nc.scalar.copy(warm[0:1, 1:2], const_sb[0:1, 0:1])
        nc.gpsimd.tensor_copy(warm[0:1, 2:3], const_sb[0:1, 0:1])

        qoff = 0
        for st, q in enumerate(QS):
            cells = 128 * q
            c0 = 128 * qoff  # starting cell = partition0's first cell offset... (layout below)

            # ---- load: partition p holds cells [c0 + q*p, c0 + q*p + q) ----
            in_t = in_pool.tile([128, QS[-1], A * NUM_PRED], f32, tag="in")
            nc.gpsimd.dma_start(
                out=in_t[:, 0:q, :],
                in_=x_d[c0 : c0 + cells, :].rearrange("(p q) c -> p q c", p=128),
            )
            in_v = in_t[:, 0:q, :].rearrange("p q (a c) -> p q a c", a=A, c=NUM_PRED)

            # exp(wh) = sigmoid(wh)/sigmoid(-wh); sgnw reads raw wh so it runs
            # before the in-place sigmoid (same engine keeps them ordered)
            sgnw = sm_pool.tile([128, QS[-1], A, 2], f32, tag="sgnw")
            nc.scalar.activation(
                sgnw[:, 0:q, :, :], in_v[:, :, :, 2:4], AF.Sigmoid, scale=-1.0
            )
            # sigmoid of everything, in place
            nc.scalar.activation(in_t[:, 0:q, :], in_t[:, 0:q, :], AF.Sigmoid)
            sig_v = in_v

            rec = sm_pool.tile([128, QS[-1], A, 2], f32, tag="rec")
            nc.vector.reciprocal(rec[:, 0:q, :, :], sgnw[:, 0:q, :, :])
            t1 = sm_pool.tile([128, QS[-1], A, 2], f32, tag="t1")
            nc.vector.tensor_tensor(
                t1[:, 0:q, :, :],
                sig_v[:, :, :, 2:4],
                hanch_v.to_broadcast([128, q, A, 2]),
                OP.mult,
            )
            halfwh = sm_pool.tile([128, QS[-1], A, 2], f32, tag="halfwh")
            nc.vector.tensor_tensor(
                halfwh[:, 0:q, :, :], t1[:, 0:q, :, :], rec[:, 0:q, :, :], OP.mult
            )

            # xy = sigmoid(xy)*stride + off*stride
            xy = sm_pool.tile([128, QS[-1], A, 2], f32, tag="xy")
            nc.vector.scalar_tensor_tensor(
                xy[:, 0:q, :, :],
                in0=sig_v[:, :, :, 0:2],
                scalar=stride_f,
                in1=const_sb[:, qoff * A * 2 : (qoff + q) * A * 2].rearrange(
                    "p (q a k) -> p q a k", a=A, k=2
                ),
                op0=OP.mult,
                op1=OP.add,
            )

            # bbox cell-major: [p, 1, q, a, 0:2]=xy-halfwh, [2:4]=xy+halfwh
            bb = sm_pool.tile([128, 1, QS[-1], A, 4], f32, tag="bb")
            nc.vector.tensor_tensor(
                bb[:, 0, 0:q, :, 0:2], xy[:, 0:q, :, :], halfwh[:, 0:q, :, :], OP.subtract
            )
            nc.vector.tensor_tensor(
                bb[:, 0, 0:q, :, 2:4], xy[:, 0:q, :, :], halfwh[:, 0:q, :, :], OP.add
            )

            # class scores S[p, q, a, c] = sigmoid(cls) * sigmoid(obj)
            S = s_pool.tile([128, QS[-1], A, C], f32, tag="S")
            nc.gpsimd.tensor_tensor(
                S[:, 0:q, :, :],
                sig_v[:, :, :, 5:85],
                sig_v[:, :, :, 4:5].to_broadcast([128, q, A, C]),
                OP.mult,
            )
            S_cqa = S[:, 0:q, :, :].rearrange("p q a c -> p c (q a)")

            # ---- per class-half: assemble + store ----
            for h, pool in ((0, oa_pool), (1, ob_pool)):
                cl = h * CHALF
                ov = pool.tile([128, CHALF, QS[-1], A, 6], f32, tag=f"ov{h}")
                ov_col = ov[:, :, 0:q, :, :].rearrange("p c q a e -> p c (q a) e")

                # score = (S > t) * S
                nc.vector.scalar_tensor_tensor(
                    ov_col[:, :, :, 1],
                    in0=S_cqa[:, cl : cl + CHALF, :],
                    scalar=THRESH,
                    in1=S_cqa[:, cl : cl + CHALF, :],
                    op0=OP.is_gt,
                    op1=OP.mult,
                )
                # cid+1 = (S > t) * (c+1); ScalarE applies the -1
                nc.vector.scalar_tensor_tensor(
                    ov_col[:, :, :, 0],
                    in0=S_cqa[:, cl : cl + CHALF, :],
                    scalar=THRESH,
                    in1=cpat_v[:, cl : cl + CHALF, :].to_broadcast([128, CHALF, q * A]),
                    op0=OP.is_gt,
                    op1=OP.mult,
                )
                nc.scalar.activation(
                    ov_col[:, :, :, 0], ov_col[:, :, :, 0], AF.Copy, bias=-1.0
                )

                # bbox broadcast across classes, per anchor, DVE/ScalarE split
                for a in range(A):
                    src = bb[:, :, 0:q, a, :]
                    nc.vector.tensor_copy(
                        ov[:, 0:BSP, 0:q, a, 2:6],
                        src.to_broadcast([128, BSP, q, 4]),
                    )
                    nc.scalar.copy(
                        ov[:, BSP:CHALF, 0:q, a, 2:6],
                        src.to_broadcast([128, CHALF - BSP, q, 4]),
                    )

                # store: DRAM (p, c, k) = (cl+c)*115200 + c0*18 + p*q*18 + k
                nc.sync.dma_start(
                    out=out_d[
                        cl : cl + CHALF, c0 * ROW : (c0 + cells) * ROW
                    ].rearrange("c (p k) -> p c k", p=128),
                    in_=ov[:, :, 0:q, :, :].rearrange("p c q a e -> p c (q a e)"),
                )
            qoff += q

    nc.finalize()
    return nc


def make_consts(anchor, offset, stride_f):
    """Pack [offs | hanch | cpat] into one (128, F) f32 blob."""
    off = np.asarray(offset, dtype=np.float32).reshape(-1, 2)[:HW_CELLS] * stride_f
    # offs[p, qoff+i, a, k] = off[c0 + p*q + i, k] for supertile at (c0, q)
    cols = []
    c0 = 0
    for q in QS:
        o = off[c0 : c0 + 128 * q].reshape(128, q, 1, 2)
        cols.append(np.broadcast_to(o, (128, q, NUM_ANCHOR, 2)).reshape(128, -1))
        c0 += 128 * q
    offs_cols = np.ascontiguousarray(np.concatenate(cols, axis=1))
    a2 = np.asarray(anchor, dtype=np.float32).reshape(NUM_ANCHOR * 2)
    hanch = np.tile(a2 / 2.0, (128, 1)).astype(np.float32)   # (128, 6)
    cpat = np.tile(np.arange(1, NUM_CLASSES + 1, dtype=np.float32), (128, 1))
    blob = np.concatenate([offs_cols, hanch, cpat], axis=1)
    return np.ascontiguousarray(blob.astype(np.float32))


def _host_prep(output, anchor, offset, stride):
    stride_f = float(stride)
    B = output.shape[0]
    x_all = np.ascontiguousarray(
        np.asarray(output, dtype=np.float32).reshape(B, HW_CELLS, NUM_ANCHOR * NUM_PRED)
    )
    consts = make_consts(anchor, offset, stride_f)
    return stride_f, x_all, consts


def kernel(output, anchor, offset, stride):
    from concourse.bass_utils import run_bass_kernel_spmd

    stride_f, x_all, consts = _host_prep(output, anchor, offset, stride)
    key = ("nc", stride_f)
    if key not in _CACHE:
        _CACHE[key] = _build(stride_f)
    nc = _CACHE[key]

    in_maps = [{"x": x_all[b], "consts": consts} for b in range(N_CORES)]
    res = run_bass_kernel_spmd(
        nc,
        in_maps,
        list(range(N_CORES)),
        tmpdir=os.environ.get("KERNEL_TRACE_DIR") or None,
    )
    global LAST_RESULT
    LAST_RESULT = res
    outs = [
        r["out"].reshape(NUM_CLASSES * HW_CELLS * NUM_ANCHOR, 6) for r in res.results
    ]
    return np.stack(outs, axis=0)


if __name__ == "__main__":
    rng = np.random.default_rng(0)
    out = rng.standard_normal((8, 80, 80, 255), dtype=np.float32)
    anchor = rng.uniform(10.0, 120.0, (1, 1, 3, 2)).astype(np.float32)
    gy, gx = np.meshgrid(np.arange(80, dtype=np.float32), np.arange(80, dtype=np.float32), indexing="ij")
    offset = np.stack([gx, gy], axis=-1).reshape(1, 80, 80, 1, 2)
    r = kernel(out, anchor, offset, 8)
    print(r.shape, r.dtype)
